# revision 12
# baseline (speedup 1.0000x reference)
"""Complex Conv1D (VALID, stride 1) on Trainium2 — Bass/Tile, 8-core data-parallel.

Problem (hardcoded shapes):
  x_real/x_imag: [32, 4096, 64] f32, kernel_real/imag: [9, 64, 64] f32,
  bias_real/imag: [64] f32  ->  out [32, 4088, 64, 2] f32
  out_real = conv(xr, wr) - conv(xi, wi) + br
  out_imag = conv(xr, wi) + conv(xi, wr) + bi

Mapping: complex multiply as its 2x2 real block-matrix form so each tap is ONE
full 128-contract matmul:
  X_b [128, L]   rows 0:64 = xr[b].T (channels on partitions), 64:128 = xi[b].T
  W[k] [128,128] = [[wr[k], wi[k]], [-wi[k], wr[k]]]
  psum[128, T] += W[k].T @ X_b[:, l0+k : l0+k+T]   for k = 0..8
  psum rows 0:64 = real output (filters), rows 64:128 = imag output.
Batch is sharded 4-per-core across 8 cores; weights replicated. The kernel
emits the output transposed as [b, 128, L_out]; the host restores
[B, L_out, F, 2].

PE does 9 rows (128x128 MACs each) per output position — 9*4088*4 = 147k rows
per core = 61.4us at 2.4GHz, the hard floor. The rest of the design keeps the
PE near that floor (HW-measured choices, each A/B'd via a repeat-loop diff):
  - bf16 operands (same 1 cycle/row as f32r, half the SBUF/DMA traffic;
    rel err ~2.3e-3 vs the f32 reference, gate is 2e-2). f32 outputs.
  - whole-batch X tiles DMA'd in 8 chunks on the SP queue: spreading the
    transfers reduced measured DMA<->PE SBUF contention vs one big burst.
  - evacuation psum->SBUF on the DVE (vector) engine, not Act: measured
    ~5us less PE interference; out-DMAs ride the Act queue; bias load on
    the gpsimd/SWDGE path to keep the startup HWDGE queue clear.
  - first X chunk small (512 cols) so the first matmul group starts ~3us
    in; warmup matmuls measured net-negative (they delay the real stream
    more than the p-state ramp costs), so warmup defaults to 0.
  - 6 PSUM banks cycling; 3 X buffers / 4 out buffers for prefetch depth.
"""

import numpy as np

import concourse.bacc as bacc
import concourse.bass as bass
import concourse.mybir as mybir
from concourse.tile import TileContext
from concourse.bass_utils import run_bass_kernel_spmd

B, L, CIN, KT, F = 32, 4096, 64, 9, 64
LOUT = L - KT + 1  # 4088
NCORES = 8
BPC = B // NCORES  # batches per core
TL = 512  # output-tile width (one PSUM bank of fp32)
NLT = (LOUT + TL - 1) // TL  # 8

MM_DT_NAME = "bfloat16"
OUT_DT_NAME = "float32"


def _build_nc(
    mm_dt,
    w_dt=None,
    out_dt=None,
    xbufs=3,
    obufs=4,
    psbufs=6,
    warmup=0,
    warmup_rows=512,
    xchunks=8,
    first_chunk=512,
    xeng="sync",
    out_eng="scalar",
    bias_eng="gpsimd",
    tail=0,
    korder=0,
    evac="dve",
    repeat=1,
    loop_repeat=None,
):
    nc = bacc.Bacc("TRN2", target_bir_lowering=False, debug=False, num_devices=NCORES)
    if w_dt is None:
        w_dt = mm_dt
    if out_dt is None:
        out_dt = getattr(mybir.dt, OUT_DT_NAME)

    x_d = nc.dram_tensor("x", [BPC, 128, L], mm_dt, kind="ExternalInput")
    w_d = nc.dram_tensor("w", [128, KT * 128], w_dt, kind="ExternalInput")
    bias_d = nc.dram_tensor("bias", [128, 1], mybir.dt.float32, kind="ExternalInput")
    out_d = nc.dram_tensor("out", [BPC, 128, LOUT], out_dt, kind="ExternalOutput")

    f32 = mybir.dt.float32
    ident = mybir.ActivationFunctionType.Identity
    oeng = getattr(nc, out_eng)
    beng = getattr(nc, bias_eng)
    xe = getattr(nc, xeng)

    with TileContext(nc) as tc:
        with (
            tc.tile_pool(name="wpool", bufs=1) as wpool,
            tc.tile_pool(name="xpool", bufs=xbufs) as xpool,
            tc.tile_pool(name="opool", bufs=obufs) as opool,
            tc.tile_pool(name="pspool", bufs=psbufs, space="PSUM") as pspool,
        ):
            # SP queue carries wt first (warmup fodder), then all x loads.
            wt = wpool.tile([128, KT * 128], w_dt)
            nc.sync.dma_start(wt[:], w_d[:])
            bias_t = wpool.tile([128, 1], f32)
            beng.dma_start(bias_t[:], bias_d[:])

            if warmup:
                # Hold the PE busy (p-state ramp) from wt-ready until the
                # first x chunk lands. With korder all 8 banks cycle in the
                # main pool, so borrow from it instead of pinning a bank.
                if korder:
                    wps = pspool.tile([128, 512], f32, tag="ps")
                else:
                    wps = pspool.tile([128, 512], f32, tag="wps", bufs=1)
                for _ in range(warmup):
                    nc.tensor.matmul(
                        wps[:, :warmup_rows], wt[:, 0:128], wt[:, 0:warmup_rows],
                        start=True, stop=True, skip_group_check=True,
                    )

            import contextlib

            loop_cm = (
                tc.For_i(0, loop_repeat, 1)
                if loop_repeat is not None
                else contextlib.nullcontext()
            )
            n_evac = 0
            with loop_cm:
              for _rep in range(repeat):
                for b in range(BPC):
                    # Whole-batch X in SBUF; DMA'd in chunks so the first
                    # tile's matmuls only wait on chunk 0.
                    xt = xpool.tile([128, L], mm_dt, tag="xt")
                    cuts = [0, first_chunk] if (b == 0 and first_chunk) else [0]
                    rest = (L - cuts[-1]) // max(1, xchunks - len(cuts) + 1)
                    while cuts[-1] + rest < L:
                        cuts.append(cuts[-1] + rest)
                    cuts.append(L)
                    for c0, c1 in zip(cuts[:-1], cuts[1:]):
                        xe.dma_start(xt[:, c0:c1], x_d[b, :, c0:c1])
                    # Tile bounds; optionally thin final tile to cut the tail.
                    bounds = [(j * TL, min(TL, LOUT - j * TL)) for j in range(NLT)]
                    if tail and b == BPC - 1:
                        l0, t = bounds[-1]
                        bounds[-1] = (l0, t - tail)
                        bounds.append((l0 + t - tail, tail))
                    def _evac(ps, l0, t, n):
                        ot = opool.tile([128, TL], out_dt, tag="ot")
                        use_dve = (
                            evac == "dve" or (evac == "alt" and n % 2 == 1)
                        )
                        if use_dve:
                            nc.vector.tensor_scalar_add(
                                ot[:, :t], ps[:, :t], bias_t[:]
                            )
                        else:
                            nc.scalar.activation(
                                ot[:, :t], ps[:, :t], ident, bias=bias_t[:]
                            )
                        oeng.dma_start(out_d[b, :, l0 : l0 + t], ot[:, :t])

                    if korder:
                        # tap-outer over groups of `korder` tiles: one
                        # weight load serves the whole group (4x fewer
                        # LdWeights -> less SBUF read pressure).
                        for g0 in range(0, len(bounds), korder):
                            grp = bounds[g0 : g0 + korder]
                            pss = []
                            for _ in grp:
                                ps_j = pspool.tile([128, TL], f32, tag="ps")
                                pss.append(ps_j)
                            for k in range(KT):
                                for ps_j, (l0, t) in zip(pss, grp):
                                    nc.tensor.matmul(
                                        ps_j[:, :t],
                                        wt[:, k * 128 : (k + 1) * 128],
                                        xt[:, l0 + k : l0 + k + t],
                                        start=(k == 0),
                                        stop=(k == KT - 1),
                                        skip_group_check=True,
                                    )
                            for ps_j, (l0, t) in zip(pss, grp):
                                _evac(ps_j, l0, t, n_evac)
                                n_evac += 1
                    else:
                      for l0, t in bounds:
                        ps = pspool.tile([128, TL], f32, tag="ps")
                        for k in range(KT):
                            nc.tensor.matmul(
                                ps[:, :t],
                                wt[:, k * 128 : (k + 1) * 128],
                                xt[:, l0 + k : l0 + k + t],
                                start=(k == 0),
                                stop=(k == KT - 1),
                            )
                        _evac(ps, l0, t, n_evac)
                        n_evac += 1

    nc.compile()
    return nc


def _pack(x_real, x_imag, kernel_real, kernel_imag, bias_real, bias_imag, np_dt,
          w_np_dt=None):
    if w_np_dt is None:
        w_np_dt = np_dt
    X = np.empty((B, 128, L), np_dt)
    X[:, :CIN] = x_real.transpose(0, 2, 1)
    X[:, CIN:] = x_imag.transpose(0, 2, 1)
    Wk = np.empty((KT, 128, 128), np.float32)
    Wk[:, :CIN, :F] = kernel_real
    Wk[:, :CIN, F:] = kernel_imag
    Wk[:, CIN:, :F] = -kernel_imag
    Wk[:, CIN:, F:] = kernel_real
    W2 = Wk.transpose(1, 0, 2).reshape(128, KT * 128).astype(w_np_dt)
    bias2 = (
        np.concatenate([bias_real, bias_imag]).reshape(128, 1).astype(np.float32)
    )
    return X, np.ascontiguousarray(W2), bias2


def _parse_dt(name):
    name = name or MM_DT_NAME
    if "," in name:
        xn, wn = name.split(",")
    else:
        xn = wn = name
    return getattr(mybir.dt, xn), getattr(mybir.dt, wn)


def _prepare(inputs, mm_dt_name=None, out_dt_name=None, build_kw=None):
    mm_dt, w_dt = _parse_dt(mm_dt_name)
    out_dt = getattr(mybir.dt, out_dt_name or OUT_DT_NAME)
    np_dt = mybir.dt.np(mm_dt)
    w_np_dt = mybir.dt.np(w_dt)
    args = {
        k: np.asarray(inputs[k], np.float32)
        for k in (
            "x_real", "x_imag", "kernel_real", "kernel_imag", "bias_real", "bias_imag",
        )
    }
    X, W2, bias2 = _pack(np_dt=np_dt, w_np_dt=w_np_dt, **args)

    nc = _build_nc(mm_dt, w_dt=w_dt, out_dt=out_dt, **(build_kw or {}))
    in_maps = [
        {
            "x": np.ascontiguousarray(X[i * BPC : (i + 1) * BPC]),
            "w": W2,
            "bias": bias2,
        }
        for i in range(NCORES)
    ]
    return nc, in_maps


def _gather(results):
    O = np.concatenate([np.asarray(r["out"], np.float32) for r in results], axis=0)
    O = O.reshape(B, 2, F, LOUT).transpose(0, 3, 2, 1)  # [B, LOUT, F, 2]
    return np.ascontiguousarray(O, dtype=np.float32)


def _run(inputs, trace=False, mm_dt_name=None, out_dt_name=None, build_kw=None):
    nc, in_maps = _prepare(inputs, mm_dt_name, out_dt_name, build_kw)
    res = run_bass_kernel_spmd(nc, in_maps, core_ids=list(range(NCORES)), trace=trace)
    return _gather(res.results), res


def kernel(**inputs) -> np.ndarray:
    out, _ = _run(inputs, trace=False)
    return out
